# revision 1
# baseline (speedup 1.0000x reference)
"""DiT block kernel for Trainium2, 8-core SPMD, no collectives.

Sharding: core c handles batch b = c//2, query-half qh = c%2 (2048 query
tokens). Host permutes each core's x so its query tokens are rows 0..2047;
K/V are computed on-core over all 4096 rows (attention is invariant to key
order). Output gathered on host.

Per-core math (E=384, NH=6, HD=64, FF=1536):
  adaln rows = cond @ [g1|be1|a1|g2|be2|a2] + biases
  scale1 = ln1_w*(1+g1); shift1 = ln1_b*(1+g1)+be1  (same for 2)
  xhat = LN(x) -> PE transpose -> y1T = xhat_T*scale1+shift1   [E,S] bf16
  KT/QT in T-layout (Q scaled by 1/8), V token-layout with ones column
  scoresT[k,q] via row-packed head-pair matmuls (chunk c = heads 2c,2c+1)
  PS = exp(scoresT) bf16 (no max subtraction; scores are small here)
  attnT_unnorm[{d,sum},q] += V_aug^T @ PS  (row 64 = softmax denominators)
  attnT = attnT_unnorm * (1/sums) via K=1 matmul broadcast
  r1 = x + attnT^T @ (wo*alpha1)            (r1 staged in DRAM)
  y2T from LN2(r1); h1T = relu(ff1^T y2T + ff1_b)
  out = r1 + h1T^T @ (ff2*alpha2) + ff2_b*alpha2
"""

import os

os.environ.setdefault("MYCRO_LOCAL_CACHE", "1")

from contextlib import ExitStack

import numpy as np

import concourse.bacc as bacc
import concourse.mybir as mybir
from concourse.masks import make_identity
from concourse.tile import TileContext

F32 = mybir.dt.float32
BF16 = mybir.dt.bfloat16
AF = mybir.ActivationFunctionType
OP = mybir.AluOpType

E = 384
NH = 6
HD = 64
FF = 1536
EPS = 1e-5
NCH = E // 128
NFH = FF // 128
NPAIR = NH // 2


def build_kernel(S_kv=4096, S_q=2048, phases=7):
    """Build the per-core Bass module. phases<7 truncates for debugging and
    dumps the newest intermediate into `out` (rest stays zero)."""
    nc = bacc.Bacc("TRN2", target_bir_lowering=False)

    NKT = S_kv // 128
    NTT = S_kv // 128
    NQT = S_q // 128
    QCH = 512 if S_q % 512 == 0 else S_q
    NQN = S_q // QCH
    NV = S_kv // 512 if S_kv % 512 == 0 else 1
    KCH = S_kv // NV

    xp = nc.dram_tensor("xp", [S_kv, E], F32, kind="ExternalInput")[:, :]
    cond_col = nc.dram_tensor("cond_col", [E, 1], F32, kind="ExternalInput")[:, :]
    adaln_w = nc.dram_tensor("adaln_w", [E, 6 * E], F32, kind="ExternalInput")[:, :]
    adaln_b = nc.dram_tensor("adaln_b", [1, 6 * E], F32, kind="ExternalInput")[:, :]
    ln1w_d = nc.dram_tensor("ln1w", [1, E], F32, kind="ExternalInput")[:, :]
    ln1b_d = nc.dram_tensor("ln1b", [1, E], F32, kind="ExternalInput")[:, :]
    ln2w_d = nc.dram_tensor("ln2w", [1, E], F32, kind="ExternalInput")[:, :]
    ln2b_d = nc.dram_tensor("ln2b", [1, E], F32, kind="ExternalInput")[:, :]
    wq_d = nc.dram_tensor("wq", [E, E], F32, kind="ExternalInput")[:, :]
    wk_d = nc.dram_tensor("wk", [E, E], F32, kind="ExternalInput")[:, :]
    wv_d = nc.dram_tensor("wv", [E, E], F32, kind="ExternalInput")[:, :]
    wo_d = nc.dram_tensor("wo", [E, E], F32, kind="ExternalInput")[:, :]
    ff1_d = nc.dram_tensor("ff1", [E, FF], F32, kind="ExternalInput")[:, :]
    ff1b_d = nc.dram_tensor("ff1b", [1, FF], F32, kind="ExternalInput")[:, :]
    ff2_d = nc.dram_tensor("ff2", [FF, E], F32, kind="ExternalInput")[:, :]
    ff2b_d = nc.dram_tensor("ff2b", [1, E], F32, kind="ExternalInput")[:, :]
    out_d = nc.dram_tensor("out", [S_q, E], F32, kind="ExternalOutput")[:, :]

    ctx = ExitStack()
    with TileContext(nc) as tc, ctx:
        root = ctx.enter_context(tc.tile_pool(name="root", bufs=1))
        dump_pool = ctx.enter_context(tc.tile_pool(name="dmp", bufs=2))

        def dump(ap, row0):
            nr, nco = ap.shape[0], min(ap.shape[-1], E)
            ap = ap[..., 0:nco]
            if len(ap.shape) == 3:
                ap = ap[:, 0, :]
            dt_ = dump_pool.tile([128, E], F32, tag="dt", name="dt")
            nc.vector.tensor_copy(dt_[0:nr, 0:nco], ap)
            nc.sync.dma_start(out_d[row0:row0 + nr, 0:nco], dt_[0:nr, 0:nco])

        ident = root.tile([128, 128], BF16)
        make_identity(nc, ident)
        ones_f = root.tile([1, 128], F32)
        nc.vector.memset(ones_f, 1.0)
        ones_bf = root.tile([1, 128], BF16)
        nc.vector.memset(ones_bf, 1.0)
        eps_t = root.tile([128, 1], F32)
        nc.vector.memset(eps_t, EPS)

        ln1w = root.tile([1, E], F32); nc.sync.dma_start(ln1w, ln1w_d)
        ln1b = root.tile([1, E], F32); nc.sync.dma_start(ln1b, ln1b_d)
        ln2w = root.tile([1, E], F32); nc.sync.dma_start(ln2w, ln2w_d)
        ln2b = root.tile([1, E], F32); nc.sync.dma_start(ln2b, ln2b_d)
        adab = root.tile([1, 6 * E], F32); nc.sync.dma_start(adab, adaln_b)
        ff2b_r = root.tile([1, E], F32); nc.sync.dma_start(ff2b_r, ff2b_d)
        cond_sb = root.tile([128, NCH, 1], F32)
        nc.sync.dma_start(cond_sb, cond_col.rearrange("(c p) o -> p c o", p=128))
        cond_bf = root.tile([128, NCH, 1], BF16)
        nc.vector.tensor_copy(cond_bf, cond_sb)

        # ---------- phase 0: AdaLN projections ----------
        adaln_rows = root.tile([1, 6, E], F32)  # g1 be1 a1 g2 be2 a2
        with tc.tile_pool(name="ph0", bufs=1) as p0, \
             tc.tile_pool(name="ph0p", bufs=2, space="PSUM") as p0p:
            aw = p0.tile([128, NCH, 6 * E], F32)
            nc.sync.dma_start(aw, adaln_w.rearrange("(c p) n -> p c n", p=128))
            aw_bf = p0.tile([128, NCH, 6 * E], BF16)
            nc.vector.tensor_copy(aw_bf, aw)
            for j in range(6):
                ps = p0p.tile([1, E], F32, tag="adps", name="adps")
                for k in range(NCH):
                    nc.tensor.matmul(ps, cond_bf[:, k, :],
                                     aw_bf[:, k, j * E:(j + 1) * E],
                                     start=(k == 0), stop=(k == NCH - 1))
                nc.vector.tensor_tensor(adaln_rows[:, j, :], ps,
                                        adab[:, j * E:(j + 1) * E], OP.add)

        g1p = root.tile([1, E], F32)
        nc.vector.tensor_scalar(g1p, adaln_rows[:, 0, :], 1.0, None, OP.add)
        g2p = root.tile([1, E], F32)
        nc.vector.tensor_scalar(g2p, adaln_rows[:, 3, :], 1.0, None, OP.add)
        scale1_r = root.tile([1, E], F32)
        nc.vector.tensor_tensor(scale1_r, g1p, ln1w, OP.mult)
        scale2_r = root.tile([1, E], F32)
        nc.vector.tensor_tensor(scale2_r, g2p, ln2w, OP.mult)
        shift1_r = root.tile([1, E], F32)
        nc.vector.tensor_tensor(shift1_r, g1p, ln1b, OP.mult)
        nc.vector.tensor_tensor(shift1_r, shift1_r, adaln_rows[:, 1, :], OP.add)
        shift2_r = root.tile([1, E], F32)
        nc.vector.tensor_tensor(shift2_r, g2p, ln2b, OP.mult)
        nc.vector.tensor_tensor(shift2_r, shift2_r, adaln_rows[:, 4, :], OP.add)

        scale1_c = root.tile([128, NCH], F32)
        shift1_c = root.tile([128, NCH], F32)
        scale2_c = root.tile([128, NCH], F32)
        shift2_c = root.tile([128, NCH], F32)
        for c in range(NCH):
            s = slice(c * 128, (c + 1) * 128)
            nc.sync.dma_start(scale1_c[:, c:c + 1], scale1_r[:, s])
            nc.sync.dma_start(shift1_c[:, c:c + 1], shift1_r[:, s])
            nc.sync.dma_start(scale2_c[:, c:c + 1], scale2_r[:, s])
            nc.sync.dma_start(shift2_c[:, c:c + 1], shift2_r[:, s])
        ff1b_c = root.tile([128, NFH], F32)
        for c in range(NFH):
            nc.sync.dma_start(ff1b_c[:, c:c + 1], ff1b_d[:, c * 128:(c + 1) * 128])

        alpha1_b = root.tile([128, E], F32)
        alpha2_b = root.tile([128, E], F32)
        with tc.tile_pool(name="abp", bufs=2, space="PSUM") as abp:
            psa = abp.tile([128, E], F32)
            nc.tensor.matmul(psa, ones_f, adaln_rows[:, 2, :], start=True, stop=True)
            nc.vector.tensor_copy(alpha1_b, psa)
            psb_ = abp.tile([128, E], F32)
            nc.tensor.matmul(psb_, ones_f, adaln_rows[:, 5, :], start=True, stop=True)
            nc.vector.tensor_copy(alpha2_b, psb_)

        fb_bf = root.tile([1, E], BF16)
        nc.vector.tensor_tensor(fb_bf, ff2b_r, adaln_rows[:, 5, :], OP.mult)

        wo_bf = root.tile([64, NH, E], BF16)

        # ff weights loaded/cast early so they overlap LN1/QKV/attention
        pffw = ctx.enter_context(tc.tile_pool(name="pffw", bufs=1))
        ff1_bf = pffw.tile([128, NCH, FF], BF16)
        ff2_bf = pffw.tile([128, NFH, E], BF16)
        with tc.tile_pool(name="fstg", bufs=1) as fst:
            f1 = fst.tile([128, NCH, FF], F32)
            nc.sync.dma_start(f1, ff1_d.rearrange("(c p) n -> p c n", p=128))
            nc.vector.tensor_copy(ff1_bf, f1)
            f2 = fst.tile([128, NFH, E], F32)
            nc.sync.dma_start(f2, ff2_d.rearrange("(c p) n -> p c n", p=128))
            for k in range(NFH):
                nc.vector.tensor_tensor(ff2_bf[:, k, :], f2[:, k, :],
                                        alpha2_b, OP.mult)

        with ExitStack() as kv:
            pkv = kv.enter_context(tc.tile_pool(name="pkv", bufs=1))
            KT = [pkv.tile([128, S_kv], BF16, tag=f"KT{c}", name=f"KT{c}")
                  for c in range(NCH)]
            QT = [pkv.tile([128, S_q], BF16, tag=f"QT{c}", name=f"QT{c}")
                  for c in range(NCH)]
            V_sb = pkv.tile([128, NKT, NH, HD + 1], BF16)
            nc.vector.memset(V_sb[:, :, :, HD:HD + 1], 1.0)
            wq_bf = pkv.tile([128, NCH, E], BF16)
            wk_bf = pkv.tile([128, NCH, E], BF16)
            wv_bf = pkv.tile([128, NCH, E], BF16)

            with tc.tile_pool(name="wstg", bufs=1) as wst:
                for w_d, w_bf in ((wq_d, wq_bf), (wk_d, wk_bf), (wv_d, wv_bf)):
                    wf = wst.tile([128, NCH, E], F32, tag="wstage", name="wstage")
                    nc.sync.dma_start(wf, w_d.rearrange("(c p) n -> p c n", p=128))
                    nc.vector.tensor_copy(w_bf, wf)
                wof = wst.tile([64, NH, E], F32, tag="wostage", name="wostage")
                nc.sync.dma_start(wof, wo_d.rearrange("(h d) n -> d h n", h=NH))
                for h in range(NH):
                    nc.vector.tensor_tensor(wo_bf[:, h, :], wof[:, h, :],
                                            alpha1_b[0:64, :], OP.mult)

            # ---------- phases 1-2: LN1 + transpose + modulate; QKV ----------
            with ExitStack() as y1s:
                py1 = y1s.enter_context(tc.tile_pool(name="y1", bufs=1))
                y1T = [py1.tile([128, S_kv], BF16, tag=f"y1T{c}", name=f"y1T{c}")
                       for c in range(NCH)]
                with tc.tile_pool(name="ln1", bufs=4) as pln, \
                     tc.tile_pool(name="ln1p", bufs=3, space="PSUM") as plnp:
                    for i in range(NTT):
                        xt = pln.tile([128, E], F32, tag="xt", name="xt")
                        nc.sync.dma_start(xt, xp[i * 128:(i + 1) * 128, :])
                        st = pln.tile([128, 6], F32, tag="st", name="st")
                        nc.vector.bn_stats(st, xt)
                        mv = pln.tile([128, 2], F32, tag="mv", name="mv")
                        nc.vector.bn_aggr(mv, st)
                        sd = pln.tile([128, 1], F32, tag="sd", name="sd")
                        nc.scalar.activation(sd, mv[:, 1:2], AF.Sqrt, bias=eps_t)
                        rstd = pln.tile([128, 1], F32, tag="rstd", name="rstd")
                        nc.vector.reciprocal(rstd, sd)
                        nmr = pln.tile([128, 1], F32, tag="nmr", name="nmr")
                        nc.vector.tensor_scalar(nmr, mv[:, 0:1], rstd, -1.0,
                                                OP.mult, OP.mult)
                        xh = pln.tile([128, E], BF16, tag="xh", name="xh")
                        nc.vector.tensor_scalar(xh, xt, rstd, nmr, OP.mult, OP.add)
                        for c in range(NCH):
                            pst = plnp.tile([128, 128], BF16, tag="pst", name="pst")
                            nc.tensor.transpose(pst, xh[:, c * 128:(c + 1) * 128],
                                                ident)
                            nc.vector.tensor_scalar(
                                y1T[c][:, i * 128:(i + 1) * 128], pst,
                                scale1_c[:, c:c + 1], shift1_c[:, c:c + 1],
                                OP.mult, OP.add)

                if phases < 2:
                    dump(y1T[0][:, 0:min(E, S_kv)], 0)
                else:
                    with tc.tile_pool(name="qkp", bufs=4, space="PSUM") as qkp, \
                         tc.tile_pool(name="vp", bufs=2, space="PSUM") as vpp:
                        for m in range(NKT):
                            ps = vpp.tile([128, E], F32, tag="v", name="v")
                            for k in range(NCH):
                                nc.tensor.matmul(ps,
                                                 y1T[k][:, m * 128:(m + 1) * 128],
                                                 wv_bf[:, k, :],
                                                 start=(k == 0), stop=(k == NCH - 1))
                            nc.vector.tensor_copy(
                                V_sb[:, m, :, 0:HD],
                                ps.rearrange("p (h d) -> p h d", h=NH))
                        for c in range(NCH):
                            for n in range(NV):
                                ps = qkp.tile([128, KCH], F32, tag="qk", name="qk")
                                for k in range(NCH):
                                    nc.tensor.matmul(
                                        ps, wk_bf[:, k, c * 128:(c + 1) * 128],
                                        y1T[k][:, n * KCH:(n + 1) * KCH],
                                        start=(k == 0), stop=(k == NCH - 1))
                                nc.vector.tensor_copy(
                                    KT[c][:, n * KCH:(n + 1) * KCH], ps)
                            for n in range(NQN):
                                ps = qkp.tile([128, QCH], F32, tag="qk", name="qk")
                                for k in range(NCH):
                                    nc.tensor.matmul(
                                        ps, wq_bf[:, k, c * 128:(c + 1) * 128],
                                        y1T[k][:, n * QCH:(n + 1) * QCH],
                                        start=(k == 0), stop=(k == NCH - 1))
                                nc.vector.tensor_scalar(
                                    QT[c][:, n * QCH:(n + 1) * QCH], ps,
                                    0.125, None, OP.mult)

            # ---------- phase 3+: attention with per-qn fused downstream ----
            if phases == 2:
                dump(KT[0][:, 0:min(E, S_kv)], 0)
                dump(QT[0][:, 0:min(E, S_q)], 128)
            if phases >= 3:
                HB = 512
                TQ = QCH // 128
                with tc.tile_pool(name="ps_sb", bufs=2) as psb, \
                     tc.tile_pool(name="nrm", bufs=1) as nrm, \
                     tc.tile_pool(name="attq", bufs=2) as attq_p, \
                     tc.tile_pool(name="dsb", bufs=2) as dsb, \
                     tc.tile_pool(name="dwk", bufs=2) as dwk, \
                     tc.tile_pool(name="sco", bufs=2, space="PSUM") as sco, \
                     tc.tile_pool(name="acc", bufs=1, space="PSUM") as acc, \
                     tc.tile_pool(name="dsp", bufs=2, space="PSUM") as dsp:
                    for qn in range(NQN):
                        qs = slice(qn * QCH, (qn + 1) * QCH)
                        atq = attq_p.tile([64, NH, QCH], BF16, tag="atq",
                                          name="atq")
                        for p in range(NPAIR):
                            h0, h1 = 2 * p, 2 * p + 1
                            pa = acc.tile([65, 2 * HB], F32, tag="pa", name="pa")
                            for kt in range(NKT):
                                ks = slice(kt * 128, (kt + 1) * 128)
                                ss = sco.tile([128, 2 * HB], F32, tag="ss",
                                              name="ss")
                                nc.tensor.matmul(ss[:, 0:QCH], KT[p][0:64, ks],
                                                 QT[p][0:64, qs],
                                                 start=True, stop=True,
                                                 tile_position=(0, 0))
                                nc.tensor.matmul(ss[:, HB:HB + QCH],
                                                 KT[p][64:128, ks],
                                                 QT[p][64:128, qs],
                                                 start=True, stop=True,
                                                 tile_position=(64, 0))
                                ex = psb.tile([128, 2 * HB], BF16, tag="ex",
                                              name="ex")
                                if QCH == HB:
                                    nc.scalar.activation(ex, ss, AF.Exp)
                                else:
                                    nc.scalar.activation(ex[:, 0:QCH],
                                                         ss[:, 0:QCH], AF.Exp)
                                    nc.scalar.activation(ex[:, HB:HB + QCH],
                                                         ss[:, HB:HB + QCH],
                                                         AF.Exp)
                                nc.tensor.matmul(pa[:, 0:QCH], V_sb[:, kt, h0, :],
                                                 ex[:, 0:QCH],
                                                 start=(kt == 0),
                                                 stop=(kt == NKT - 1))
                                nc.tensor.matmul(pa[:, HB:HB + QCH],
                                                 V_sb[:, kt, h1, :],
                                                 ex[:, HB:HB + QCH],
                                                 start=(kt == 0),
                                                 stop=(kt == NKT - 1))
                            ta = nrm.tile([65, 2 * HB], F32, tag="ta", name="ta")
                            nc.vector.tensor_copy(ta[:, 0:QCH], pa[:, 0:QCH])
                            nc.vector.tensor_copy(ta[:, HB:HB + QCH],
                                                  pa[:, HB:HB + QCH])
                            sums = nrm.tile([1, 2 * HB], F32, tag="sums",
                                            name="sums")
                            nc.sync.dma_start(sums[:, 0:QCH], ta[64:65, 0:QCH])
                            nc.sync.dma_start(sums[:, HB:HB + QCH],
                                              ta[64:65, HB:HB + QCH])
                            rrow = nrm.tile([1, 2 * HB], F32, tag="rrow",
                                            name="rrow")
                            nc.vector.reciprocal(rrow[:, 0:QCH], sums[:, 0:QCH])
                            nc.vector.reciprocal(rrow[:, HB:HB + QCH],
                                                 sums[:, HB:HB + QCH])
                            rbc = sco.tile([64, 2 * HB], F32, tag="ss", name="rbc")
                            nc.tensor.matmul(rbc[:, 0:QCH], ones_f[:, 0:64],
                                             rrow[:, 0:QCH],
                                             start=True, stop=True)
                            nc.tensor.matmul(rbc[:, HB:HB + QCH], ones_f[:, 0:64],
                                             rrow[:, HB:HB + QCH],
                                             start=True, stop=True)
                            nc.vector.tensor_tensor(atq[:, h0, :],
                                                    ta[0:64, 0:QCH],
                                                    rbc[:, 0:QCH], OP.mult)
                            nc.vector.tensor_tensor(atq[:, h1, :],
                                                    ta[0:64, HB:HB + QCH],
                                                    rbc[:, HB:HB + QCH], OP.mult)

                        if phases == 3:
                            if qn == 0:
                                dump(atq[:, 0, 0:min(E, QCH)], 0)
                                dump(atq[:, 1, 0:min(E, QCH)], 64)
                            continue

                        # ---- fused downstream for this q-chunk ----
                        r1q = dsb.tile([128, TQ, E], F32, tag="r1q", name="r1q")
                        y2q = dsb.tile([128, NCH, QCH], BF16, tag="y2q",
                                       name="y2q")
                        for t in range(TQ):
                            trow = qn * QCH + t * 128
                            ps = dsp.tile([128, E], F32, tag="dsp", name="wops")
                            for h in range(NH):
                                nc.tensor.matmul(
                                    ps, atq[:, h, t * 128:(t + 1) * 128],
                                    wo_bf[:, h, :],
                                    start=(h == 0), stop=(h == NH - 1))
                            xq = dwk.tile([128, E], F32, tag="xq", name="xq")
                            nc.sync.dma_start(xq, xp[trow:trow + 128, :])
                            nc.vector.tensor_tensor(r1q[:, t, :], ps, xq, OP.add)
                            # LN2 on this tile
                            st = dwk.tile([128, 6], F32, tag="st2", name="st2")
                            nc.vector.bn_stats(st, r1q[:, t, :])
                            mv = dwk.tile([128, 2], F32, tag="mv2", name="mv2")
                            nc.vector.bn_aggr(mv, st)
                            sd = dwk.tile([128, 1], F32, tag="sd2", name="sd2")
                            nc.scalar.activation(sd, mv[:, 1:2], AF.Sqrt,
                                                 bias=eps_t)
                            rstd = dwk.tile([128, 1], F32, tag="rstd2",
                                            name="rstd2")
                            nc.vector.reciprocal(rstd, sd)
                            nmr = dwk.tile([128, 1], F32, tag="nmr2", name="nmr2")
                            nc.vector.tensor_scalar(nmr, mv[:, 0:1], rstd, -1.0,
                                                    OP.mult, OP.mult)
                            xh = dwk.tile([128, E], BF16, tag="xh2", name="xh2")
                            nc.vector.tensor_scalar(xh, r1q[:, t, :], rstd, nmr,
                                                    OP.mult, OP.add)
                            for c in range(NCH):
                                pst = dsp.tile([128, 128], BF16, tag="dsp",
                                               name="pst2")
                                nc.tensor.transpose(
                                    pst, xh[:, c * 128:(c + 1) * 128], ident)
                                nc.vector.tensor_scalar(
                                    y2q[:, c, t * 128:(t + 1) * 128], pst,
                                    scale2_c[:, c:c + 1], shift2_c[:, c:c + 1],
                                    OP.mult, OP.add)

                        h1q = dsb.tile([128, NFH, QCH], BF16, tag="h1q",
                                       name="h1q")
                        for m in range(NFH):
                            ps = dsp.tile([128, QCH], F32, tag="dsp", name="f1ps")
                            for k in range(NCH):
                                nc.tensor.matmul(
                                    ps, ff1_bf[:, k, m * 128:(m + 1) * 128],
                                    y2q[:, k, :],
                                    start=(k == 0), stop=(k == NCH - 1))
                            nc.vector.tensor_scalar(
                                h1q[:, m, :], ps,
                                ff1b_c[:, m:m + 1], 0.0, OP.add, OP.max)

                        for t in range(TQ):
                            trow = qn * QCH + t * 128
                            ps = dsp.tile([128, E], F32, tag="dsp", name="f2ps")
                            for k in range(NFH):
                                nc.tensor.matmul(
                                    ps, h1q[:, k, t * 128:(t + 1) * 128],
                                    ff2_bf[:, k, :],
                                    start=(k == 0), stop=False)
                            nc.tensor.matmul(ps, ones_bf, fb_bf,
                                             start=False, stop=True)
                            ot = dwk.tile([128, E], F32, tag="ot", name="ot")
                            nc.vector.tensor_tensor(ot, ps, r1q[:, t, :], OP.add)
                            nc.sync.dma_start(out_d[trow:trow + 128, :], ot)

    nc.finalize()
    return nc


_NC_CACHE = {}


def _get_nc(S_kv, S_q):
    key = (S_kv, S_q)
    if key not in _NC_CACHE:
        _NC_CACHE[key] = build_kernel(S_kv, S_q)
    return _NC_CACHE[key]


def make_in_maps(inputs, n_cores=8, S=4096):
    """Shard FULL inputs into per-core input maps."""
    x = np.asarray(inputs["x"], np.float32)
    cond = np.asarray(inputs["cond"], np.float32)
    Sq = S // 2
    adaln_w = np.concatenate(
        [np.asarray(inputs[k], np.float32)
         for k in ("g1_w", "be1_w", "a1_w", "g2_w", "be2_w", "a2_w")], axis=1)
    adaln_b = np.concatenate(
        [np.asarray(inputs[k], np.float32)
         for k in ("g1_b", "be1_b", "a1_b", "g2_b", "be2_b", "a2_b")])[None, :]
    shared = {
        "adaln_w": np.ascontiguousarray(adaln_w),
        "adaln_b": np.ascontiguousarray(adaln_b),
        "ln1w": np.asarray(inputs["ln1_w"], np.float32)[None, :],
        "ln1b": np.asarray(inputs["ln1_b"], np.float32)[None, :],
        "ln2w": np.asarray(inputs["ln2_w"], np.float32)[None, :],
        "ln2b": np.asarray(inputs["ln2_b"], np.float32)[None, :],
        "wq": np.asarray(inputs["wq"], np.float32),
        "wk": np.asarray(inputs["wk"], np.float32),
        "wv": np.asarray(inputs["wv"], np.float32),
        "wo": np.asarray(inputs["wo"], np.float32),
        "ff1": np.asarray(inputs["ff1_w"], np.float32),
        "ff1b": np.asarray(inputs["ff1_b"], np.float32)[None, :],
        "ff2": np.asarray(inputs["ff2_w"], np.float32),
        "ff2b": np.asarray(inputs["ff2_b"], np.float32)[None, :],
    }
    in_maps = []
    for c in range(n_cores):
        b, qh = c // 2, c % 2
        xb = x[b]
        xpm = np.concatenate([xb[qh * Sq:(qh + 1) * Sq],
                              xb[(1 - qh) * Sq:(2 - qh) * Sq]], axis=0)
        m = dict(shared)
        m["xp"] = np.ascontiguousarray(xpm)
        m["cond_col"] = np.ascontiguousarray(cond[b].reshape(E, 1))
        in_maps.append(m)
    return in_maps


def kernel(**inputs):
    from concourse.bass_utils import run_bass_kernel_spmd

    x = np.asarray(inputs["x"], np.float32)
    B, S, _ = x.shape
    Sq = S // 2
    nc = _get_nc(S, Sq)
    in_maps = make_in_maps(inputs, n_cores=8, S=S)
    res = run_bass_kernel_spmd(nc, in_maps, core_ids=list(range(8)))
    out = np.empty((B, S, E), np.float32)
    for c in range(8):
        b, qh = c // 2, c % 2
        out[b, qh * Sq:(qh + 1) * Sq] = res.results[c]["out"]
    return out

